# revision 5
# baseline (speedup 1.0000x reference)
"""Block-causal attention TRN2 kernel (8-core SPMD, head-sharded).

Problem: y = (softmax(mask(Q K^T / sqrt(d))) V) W_out + b_out where
Q,K,V = x W_qkv + b_qkv, x [2, 2048, 1024], 16 heads of d=64, block-causal
mask with chunk 128.

Sharding: core c handles batch b = c//4 and head group g = c%4 (4 heads).
Each core computes its heads' QKV projection (W_qkv column slice), the
block-causal attention, and a partial out-projection against its W_out row
slice. The host sums the 4 partial outputs per batch and adds b_out.

On-device layout is "transposed": Q^T/K^T [d, L] tiles feed scores^T
matmuls (2 heads packed into the 128-partition contraction dim via PE row
groups), exp runs on the scalar engine with the 1/sqrt(d) scale folded in,
attn@V accumulates with an extra ones-column of V producing the softmax
denominators, and the normalized o^T directly feeds the out-projection as
the stationary operand. All matmul operands are float32r (~2e-4 rel err at
bf16 speed). The softmax reciprocal row is broadcast across partitions with
a K=1 PE matmul against a ones vector.

Everything runs in one pipelined loop over 512-row l/i-tiles: project tile
t, attend queries of tile t against key tiles 0..t, out-project tile t —
so PE, ACT, DVE and DMA work for different stages overlap.
"""

import sys

for _p in ("/opt/trn_rl_repo", "/root/.axon_site/_ro/trn_rl_repo"):
    if _p not in sys.path:
        sys.path.append(_p)

import numpy as np

import concourse.bass as bass
import concourse.mybir as mybir
import concourse.tile as tile
from concourse import bacc
from concourse.bass_utils import run_bass_kernel_spmd
from concourse.masks import make_identity

F32 = mybir.dt.float32
F32R = mybir.dt.float32r
EXP = mybir.ActivationFunctionType.Exp
ADD = mybir.AluOpType.add

B, L, D = 2, 2048, 1024
H, DH = 16, 64          # total heads, head dim
CHUNK = 128
HPC = 4                 # heads per core
S = HPC * DH            # 256 per-core qkv width per projection
N_CORES = 8
LT = 512                # l-tile (i-tile) size
NLT = L // LT           # 4
NKT = D // 128          # 8 k-tiles over D
NCT = 3 * S // 128      # 6 c-tiles (q pair0, q pair1, k p0, k p1, v p0, v p1)
NJT = L // CHUNK        # 16 j-tiles/chunks
SCALE = 1.0 / float(np.sqrt(DH))


def build_program():
    nc = bacc.Bacc("TRN2", target_bir_lowering=False, debug=False)
    x_d = nc.dram_tensor("x", [L, D], F32, kind="ExternalInput")
    w_d = nc.dram_tensor("w_qkv", [D, 3 * S], F32, kind="ExternalInput")
    bq_d = nc.dram_tensor("b_qkv", [3 * S], F32, kind="ExternalInput")
    wo_d = nc.dram_tensor("w_out", [S, D], F32, kind="ExternalInput")
    y_d = nc.dram_tensor("y", [L, D], F32, kind="ExternalOutput")

    with tile.TileContext(nc) as tc:
        lp = nc.allow_low_precision(reason="float32r matmul pipeline")
        lp.__enter__()
        with tc.tile_pool(name="const", bufs=1) as const, \
             tc.tile_pool(name="big", bufs=1) as big, \
             tc.tile_pool(name="stage", bufs=6) as stage, \
             tc.tile_pool(name="xtp", bufs=2) as xtp, \
             tc.tile_pool(name="expp", bufs=6) as expp, \
             tc.tile_pool(name="work", bufs=2) as work, \
             tc.tile_pool(name="small", bufs=2) as small, \
             tc.tile_pool(name="ps_misc", bufs=2, space="PSUM") as ps_misc, \
             tc.tile_pool(name="ps_pp", bufs=2, space="PSUM") as ps_pp, \
             tc.tile_pool(name="ps_s", bufs=2, space="PSUM") as ps_s, \
             tc.tile_pool(name="ps_o", bufs=2, space="PSUM") as ps_o:

            # ---- constants ----
            ident_f = const.tile([128, 128], F32)
            make_identity(nc, ident_f[:])
            identr = const.tile([128, 128], F32R)
            nc.vector.tensor_copy(identr[:], ident_f[:])
            ones_f = const.tile([128, 1], F32)
            nc.vector.memset(ones_f[:], 1.0)
            ones64 = const.tile([1, 64], F32R)
            o64f = const.tile([1, 64], F32)
            nc.vector.memset(o64f[:], 1.0)
            nc.vector.tensor_copy(ones64[:], o64f[:])
            # b_qkv as per-c-tile per-partition bias columns [128, 6]
            bq_sb = const.tile([128, NCT], F32)
            bq_ap = bq_d.ap()
            nc.sync.dma_start(
                out=bq_sb[:],
                in_=bass.AP(tensor=bq_ap.tensor, offset=bq_ap.offset,
                            ap=[[1, 128], [128, NCT]]),
            )

            # ---- persistent weights/activations ----
            w_sb = big.tile([128, NKT, 3 * S], F32R)       # W_qkv k-tiles
            for kt in range(NKT):
                nc.sync.dma_start(out=w_sb[:, kt, :],
                                  in_=w_d[kt * 128:(kt + 1) * 128, :].bitcast(F32R))
            wo_sb = big.tile([128, 2, D], F32R)            # W_out k-tiles (head pairs)
            for p in range(2):
                nc.sync.dma_start(out=wo_sb[:, p, :],
                                  in_=wo_d[p * 128:(p + 1) * 128, :].bitcast(F32R))
            qt_sb = big.tile([128, 2, L], F32R)            # Q^T pair-stacked
            kt_sb = big.tile([128, 2, L], F32R)            # K^T pair-stacked
            v_sb = big.tile([128, HPC, NJT, 65], F32R)     # V + ones col
            ot_sb = big.tile([128, 2, L], F32R)            # normalized o^T
            nc.vector.tensor_copy(
                v_sb[:, :, :, 64:65],
                bass.AP(tensor=ones_f.tensor, offset=ones_f.offset,
                        ap=ones_f.ap[:1] + [[0, HPC], [0, NJT], [0, 1]]),
            )

            for t in range(NLT):
                l0 = t * LT
                # ---------- stage 1: x^T for l-tile t ----------
                xs = []
                for sp in range(4):
                    xst = stage.tile([128, D], F32R, tag="xs", name=f"xs_{t}_{sp}")
                    nc.sync.dma_start(
                        out=xst[:],
                        in_=x_d[l0 + sp * 128: l0 + (sp + 1) * 128, :].bitcast(F32R))
                    xs.append(xst)
                xT = xtp.tile([128, NKT, LT], F32R, tag="xT", name=f"xT_{t}")
                for kt in range(NKT):
                    tp = ps_misc.tile([128, LT], F32R, tag="m", name=f"tp_{t}_{kt}")
                    for sp in range(4):
                        nc.tensor.transpose(
                            tp[:, sp * 128:(sp + 1) * 128],
                            xs[sp][:, kt * 128:(kt + 1) * 128], identr[:])
                    nc.vector.tensor_copy(xT[:, kt, :], tp[:])

                # ---------- stage 2: QKV projection for l-tile t ----------
                for ct in range(NCT):
                    pp = ps_pp.tile([128, LT], F32, tag="pp", name=f"pp_{t}_{ct}")
                    for kt in range(NKT):
                        nc.tensor.matmul(
                            pp[:], w_sb[:, kt, ct * 128:(ct + 1) * 128],
                            xT[:, kt, :],
                            start=(kt == 0), stop=(kt == NKT - 1))
                    if ct < 4:
                        dst = qt_sb if ct < 2 else kt_sb
                        nc.vector.tensor_scalar(
                            out=dst[:, ct % 2, l0:l0 + LT], in0=pp[:],
                            scalar1=bq_sb[:, ct:ct + 1], scalar2=None, op0=ADD)
                    else:
                        pv = ct - 4
                        vt_tmp = work.tile([128, LT], F32R, tag="vt_tmp",
                                           name=f"vt_{t}_{pv}")
                        nc.vector.tensor_scalar(
                            out=vt_tmp[:], in0=pp[:],
                            scalar1=bq_sb[:, ct:ct + 1], scalar2=None, op0=ADD)
                        tpv = ps_misc.tile([128, LT], F32R, tag="m",
                                           name=f"tpv_{t}_{pv}")
                        for sp in range(4):
                            nc.tensor.transpose(
                                tpv[:, sp * 128:(sp + 1) * 128],
                                vt_tmp[:, sp * 128:(sp + 1) * 128], identr[:])
                        # tpv = [j(128), sp(4) x (head-even 64 | head-odd 64)]
                        tpv_v = tpv[:].rearrange("j (sp h d) -> j sp h d",
                                                 sp=4, h=2)
                        for hh in range(2):
                            nc.vector.tensor_copy(
                                v_sb[:, 2 * pv + hh, 4 * t:4 * (t + 1), 0:64],
                                tpv_v[:, :, hh, :])

                # ---------- stage 3: attention for i-tile t ----------
                for p in range(2):
                    o_ps = [ps_o.tile([65, LT], F32, tag="o_ps",
                                      name=f"o_ps_{p}_{t}_{hh}") for hh in range(2)]
                    njt = 4 * (t + 1)
                    for jt in range(njt):
                        vis = max(0, jt - 4 * t) * 128
                        s_pair = [ps_s.tile([128, LT], F32, tag="s",
                                            name=f"s_{p}_{t}_{jt}_{hh}")
                                  for hh in range(2)]
                        for hh in range(2):
                            nc.tensor.matmul(
                                s_pair[hh][:, vis:LT],
                                kt_sb[hh * 64:(hh + 1) * 64, p,
                                      jt * 128:(jt + 1) * 128],
                                qt_sb[hh * 64:(hh + 1) * 64, p,
                                      l0 + vis:l0 + LT],
                                start=True, stop=True)
                        for hh in range(2):
                            h = 2 * p + hh
                            e_t = expp.tile([128, LT], F32R, tag="e_t",
                                            name=f"e_{p}_{t}_{jt}_{hh}")
                            nc.scalar.activation(
                                e_t[:, vis:LT], s_pair[hh][:, vis:LT],
                                EXP, scale=SCALE)
                            nc.tensor.matmul(
                                o_ps[hh][:, vis:LT], v_sb[:, h, jt, :],
                                e_t[:, vis:LT],
                                start=(jt == 0), stop=(jt == njt - 1))
                    # normalization: r = 1/colsum; PE-broadcast; scale rows
                    r2 = small.tile([1, 2, LT], F32R, tag="r2", name=f"r2_{p}_{t}")
                    for hh in range(2):
                        nc.vector.reciprocal(r2[:, hh, :], o_ps[hh][64:65, :])
                    for hh in range(2):
                        rb = ps_misc.tile([64, LT], F32, tag="m",
                                          name=f"rb_{p}_{t}_{hh}")
                        nc.tensor.matmul(rb[:], ones64[:], r2[:, hh, :],
                                         start=True, stop=True)
                        rb_sb = work.tile([64, LT], F32, tag="rb_sb",
                                          name=f"rbs_{p}_{t}_{hh}")
                        nc.scalar.copy(rb_sb[:], rb[:])
                        if hh == 0:
                            nc.vector.tensor_mul(
                                ot_sb[0:64, p, l0:l0 + LT],
                                o_ps[hh][0:64, :], rb_sb[:])
                        else:
                            oB = work.tile([64, LT], F32R, tag="oB",
                                           name=f"oB_{p}_{t}")
                            nc.vector.tensor_mul(oB[:], o_ps[hh][0:64, :],
                                                 rb_sb[:])
                            nc.sync.dma_start(
                                out=ot_sb[64:128, p, l0:l0 + LT], in_=oB[:])

                # ---------- stage 4: out-projection for i-tile t ----------
                for st in range(4):
                    i0 = l0 + st * 128
                    for mt in range(2):
                        yp = ps_pp.tile([128, 512], F32, tag="pp",
                                        name=f"yp_{t}_{st}_{mt}")
                        for p in range(2):
                            nc.tensor.matmul(
                                yp[:], ot_sb[:, p, i0:i0 + 128],
                                wo_sb[:, p, mt * 512:(mt + 1) * 512],
                                start=(p == 0), stop=(p == 1))
                        y_sb = work.tile([128, 512], F32, tag="y_sb",
                                         name=f"ysb_{t}_{st}_{mt}")
                        nc.vector.tensor_copy(y_sb[:], yp[:])
                        nc.sync.dma_start(
                            out=y_d[i0:i0 + 128, mt * 512:(mt + 1) * 512],
                            in_=y_sb[:])
        lp.__exit__(None, None, None)
    nc.compile()
    return nc


_NC_CACHE = {}


def _get_nc():
    if "nc" not in _NC_CACHE:
        _NC_CACHE["nc"] = build_program()
    return _NC_CACHE["nc"]


def make_in_maps(x, W_qkv, b_qkv, W_out):
    x = np.ascontiguousarray(np.asarray(x, dtype=np.float32))
    W_qkv = np.asarray(W_qkv, dtype=np.float32)
    b_qkv = np.asarray(b_qkv, dtype=np.float32)
    W_out = np.asarray(W_out, dtype=np.float32)
    in_maps = []
    for c in range(N_CORES):
        b, g = divmod(c, 4)
        cols = np.concatenate([np.arange(blk * D + g * S, blk * D + (g + 1) * S)
                               for blk in range(3)])
        in_maps.append({
            "x": np.ascontiguousarray(x[b]),
            "w_qkv": np.ascontiguousarray(W_qkv[:, cols]),
            "b_qkv": np.ascontiguousarray(b_qkv[cols]),
            "w_out": np.ascontiguousarray(W_out[g * S:(g + 1) * S, :]),
        })
    return in_maps


def kernel(x, W_qkv, b_qkv, W_out, b_out):
    nc = _get_nc()
    in_maps = make_in_maps(x, W_qkv, b_qkv, W_out)
    res = run_bass_kernel_spmd(nc, in_maps, list(range(N_CORES)))
    b_out = np.asarray(b_out, dtype=np.float32)
    out = np.zeros((B, L, D), dtype=np.float32)
    for c in range(N_CORES):
        out[c // 4] += res.results[c]["y"]
    out += b_out[None, None, :]
    return out
